# revision 45
# baseline (speedup 1.0000x reference)
"""CNLP (common-neighbor link prediction) kernel for Trainium2, 8 NeuronCores.

Reference computation (per query edge e = (i, j)):
    cn  = adj[i] * adj[j]                      # common-neighbor indicator [N]
    xcn = cn @ x                               # sum of common-neighbor feats
    xij = relu(x[i]*x[j] @ Wa.T + ba) @ Wb.T + bb
    hcn = (relu->relu->lin) 3-layer MLP on xcn
    out = (hcn * beta + xij) @ Wl.T + bl       # [E, 1]

Sharding: edges (E=8192) split 8 x 1024 across cores; adj/x/weights replicated.

Device strategy per core (1024 edges in 2 blocks of 512):
  - adj is binary -> BIT-PACKED host-side (10240 nodes -> 1280 bytes/row).
    Extended row [packed 1280B | x bf16 512B]; one gpsimd
    dma_gather(transpose=True) per (block, src).  The q7 library for the
    gather is preloaded with load_library(mlp) as the first pool
    instruction so its ~14us load overlaps the input DMAs.
  - DVE ANDs the packed pair (u16 2x mode) then EXPANDS bits to fp8 bytes
    with 8 fused shift+mask tensor_scalar ops per block:
        OUT[p, m, c, e] = shift_m(cn_packed[p, c, e]) & 0x1010
    giving fp8 byte 0x10 (=2^-5) at node 2048c + 16p + 8par + m (par = byte
    within the u16 lane).  The node permutation is absorbed into the
    host-permuted stationary x table (pre-scaled by 32: 2^-5 * 32x = x).
  - Big matmul FLIPPED: stationary = permuted fp8 x table, moving = expanded
    cn slices; PSUM accumulates xcn^T feature-major [128f, 512e].  DoubleRow
    fp8 perf mode (2 k-tiles = adjacent m-planes) for 2x PE rate.
  - Both blocks' big-MM streams run back-to-back (block1's AND/expand are
    queued on DVE before any MLP vector work); the xij 2-layer MLPs are
    interleaved INTO the MM streams; the two hcn tail chains run
    interleaved on separate act engines (scalar for b0, vector for b1).
"""

import numpy as np
import ml_dtypes

import concourse.bacc as bacc
import concourse.tile as tile
import concourse.mybir as mybir
from concourse import library_config
from concourse.bass_utils import run_bass_kernel_spmd

BF16 = mybir.dt.bfloat16
FP32 = mybir.dt.float32
FP32R = mybir.dt.float32r
FP8 = mybir.dt.float8e4
I16 = mybir.dt.int16
AF = mybir.ActivationFunctionType
ALU = mybir.AluOpType
BF16_NP = ml_dtypes.bfloat16
FP8_NP = ml_dtypes.float8_e4m3

N_CORES = 8
N, E, D, H = 10000, 8192, 256, 256
NPAD = 10240                      # n padded to a multiple of 2048
EC = E // N_CORES                 # 1024 edges per core
EB = 512                          # edges per block
NB = EC // EB                     # 2 blocks
PKB = NPAD // 8                   # 1280 packed adjacency bytes per row
ROWB = PKB + 2 * D                # 1792 bytes per extended row
NC5 = PKB // 256                  # 5 u16 word-chunks of packed bits
XSCALE = 32.0                     # x table pre-scale (cn byte is 2^-5)


def build_program():
    nc = bacc.Bacc("TRN2", target_bir_lowering=False, debug=False,
                   enable_asserts=False, num_devices=N_CORES)

    adjx = nc.dram_tensor("adjx", [N, ROWB], FP8, kind="ExternalInput")
    # permuted+scaled fp8 x table, mp-major: [p][mp][c][par][fh][t][f]
    x8n_d = nc.dram_tensor("x8n", [128, 4 * NC5 * 2 * 2 * 2 * 128], FP8,
                           kind="ExternalInput")
    idxg_d = nc.dram_tensor("idxg", [128, NB * 2 * EB // 16], I16,
                            kind="ExternalInput")
    wpack_d = nc.dram_tensor("wpack", [128, 2576], FP32, kind="ExternalInput")
    out_d = nc.dram_tensor("out", [1, EC], FP32, kind="ExternalOutput")

    with tile.TileContext(nc) as tc:
        # library load FIRST (before pool-init memsets): its ~13us q7 code
        # DMA gates the first gather and overlaps the input loads
        nc.gpsimd.load_library(library_config.mlp)
        with (
            tc.tile_pool(name="const", bufs=1) as constp,
            tc.tile_pool(name="gath", bufs=4) as gathp,
            tc.tile_pool(name="exp", bufs=8) as expp,
            tc.tile_pool(name="acts", bufs=10) as actp,
            tc.tile_pool(name="px", bufs=4, space="PSUM") as pxp,
            tc.tile_pool(name="pm", bufs=2, space="PSUM") as pmp,
            tc.tile_pool(name="po", bufs=2, space="PSUM") as pop,
        ):
            idxg_sb = constp.tile([128, NB * 2 * EB // 16], I16)
            nc.sync.dma_start(idxg_sb[:], idxg_d[:])

            # all 4 gathers upfront (pool queue; ~5us decode each, serial)
            gt = {}
            for b in range(NB):
                for s in range(2):
                    gsl = slice((2 * b + s) * EB // 16,
                                (2 * b + s + 1) * EB // 16)
                    t = gathp.tile([128, ROWB // 128, EB], FP8, tag="g",
                                   bufs=4, name=f"a{b}{s}")
                    nc.gpsimd.dma_gather(
                        t[:], adjx[:], idxg_sb[:, gsl], EB, EB,
                        elem_size=ROWB, transpose=True)
                    gt[(b, s)] = t

            # fp8 x table, split by mp quarter so early matmuls start sooner
            x8n_sb = constp.tile([128, 4, NC5, 2, 2, 2, 128], FP8)
            qsz = NC5 * 2 * 2 * 2 * 128
            for mp in range(4):
                nc.sync.dma_start(
                    x8n_sb[:, mp, :, :, :, :, :]
                    .rearrange("p c q f t g -> p (c q f t g)"),
                    x8n_d[:, mp * qsz:(mp + 1) * qsz])

            # weights: one packed DMA -> scalar-copy to fp32r (the BIR
            # verifier requires fp32r matmul operands to be fp32r-rounded)
            wpack = constp.tile([128, 2576], FP32)
            nc.sync.dma_start(wpack[:], wpack_d[:])
            w_sb = {}
            for i, nm in enumerate(("wat", "wbt", "w1t", "w2t", "w3t")):
                t = constp.tile([128, 2, H], FP32R, tag=f"w_{nm}")
                nc.scalar.activation(
                    t[:], wpack[:, i * 512:(i + 1) * 512]
                    .rearrange("p (k h) -> p k h", k=2), AF.Copy)
                w_sb[nm] = t
            wlt_sb = constp.tile([128, 2, 1], FP32R)
            nc.scalar.activation(
                wlt_sb[:], wpack[:, 2560:2562]
                .rearrange("p (k o) -> p k o", k=2), AF.Copy)
            b_sb = {}
            for i, nm in enumerate(("ba", "bb", "b1", "b2", "b3")):
                b_sb[nm] = (wpack[:, 2562 + 2 * i:2564 + 2 * i]
                            .rearrange("p (k o) -> p k o", k=2))
            bl_sb = wpack[0:1, 2572:2573]
            beta_sb = wpack[:, 2573:2574]

            out_sb = constp.tile([1, EC], FP32)

            # DVE warm-up: the first instruction of each ALU-op combo pays a
            # ~1.5-3us uop-program load; DVE is idle during the ~35us gather
            # startup, so prime every combo used later on a scrap tile.
            scrap = constp.tile([128, 32], I16)
            scrap2 = constp.tile([128, 32], FP32)
            nc.vector.tensor_tensor(scrap[:, 0:16], scrap[:, 0:16],
                                    scrap[:, 16:32], ALU.bitwise_and)
            nc.vector.tensor_scalar(scrap[:, 0:16], scrap[:, 16:32], 2,
                                    0x1010, ALU.logical_shift_left,
                                    ALU.bitwise_and)
            nc.vector.tensor_scalar(scrap[:, 0:16], scrap[:, 16:32], 2,
                                    0x1010, ALU.logical_shift_right,
                                    ALU.bitwise_and)
            nc.vector.tensor_scalar(scrap[:, 0:16], scrap[:, 16:32],
                                    0x1010, None, ALU.bitwise_and)
            nc.vector.tensor_scalar(scrap2[:, 0:16], scrap2[:, 16:32],
                                    1.0, 0.0, ALU.mult, ALU.add)
            nc.vector.tensor_scalar(scrap2[:, 0:16], scrap2[:, 16:32],
                                    0.0, 0.0, ALU.add, ALU.max)
            nc.vector.tensor_tensor(scrap2[:, 0:16], scrap2[:, 0:16],
                                    scrap2[:, 16:32], ALU.add)
            nc.vector.tensor_tensor(scrap2[:, 0:16], scrap2[:, 0:16],
                                    scrap2[:, 16:32], ALU.mult)

            # PE warm-up: absorb the first-matmul pipeline penalty (~0.5us)
            # during the idle library-load window.  Dummy DoubleRow fp8 MM
            # on the loaded x table, dummy fp32r MM on the weights; results
            # discarded (separate PSUM tiles, complete groups).
            pwu = pop.tile([128, 128], FP32, tag="po", name="pwu")
            nc.tensor.matmul(pwu[:], x8n_sb[:, 0, 0, 0, 0, :, :],
                             x8n_sb[:, 0, 0, 1, 0, :, :],
                             start=True, stop=True,
                             perf_mode=mybir.MatmulPerfMode.DoubleRow)
            pwu2 = pop.tile([128, 64], FP32, tag="po", name="pwu2")
            nc.tensor.matmul(pwu2[:], w_sb["wat"][:, 0, 0:128],
                             w_sb["wat"][:, 1, 0:64],
                             start=True, stop=True)

            # MLP layer, feature-major fp32r (fp22 reads), 512 edges.
            # act_eng: 'scalar' | 'vector' | 'mixed' (t=0 scalar, t=1 vector
            # so the two half-activations run concurrently)
            # scale: optional AP multiplied into the pre-bias value (used to
            # fold *beta into the last hcn layer; its bias is b3*beta)
            def lin_h(src, wname, bname, relu, dst, act_eng="scalar",
                      scale=None):
                w, bias = w_sb[wname], b_sb[bname]
                for t in range(2):
                    pm = pmp.tile([128, EB], FP32, tag="pm")
                    for k in range(2):
                        nc.tensor.matmul(
                            pm[:], w[:, k, t * 128:(t + 1) * 128],
                            src[:, k, :], start=(k == 0), stop=(k == 1))
                    dsl = dst[:, t, :]
                    use_scalar = (act_eng == "scalar"
                                  or (act_eng == "mixed" and t == 0))
                    if use_scalar:
                        nc.scalar.activation(
                            dsl, pm[:], AF.Relu if relu else AF.Identity,
                            bias=bias[:, t, :],
                            scale=scale if scale is not None else 1.0)
                    elif scale is not None:
                        nc.vector.tensor_scalar(
                            dsl, pm[:], scale, bias[:, t, :],
                            ALU.mult, ALU.add)
                    elif relu:
                        nc.vector.tensor_scalar(
                            dsl, pm[:], bias[:, t, :], 0.0,
                            ALU.add, ALU.max)
                    else:
                        nc.vector.tensor_scalar_add(dsl, pm[:], bias[:, t, :])
                return dst

            # ---- per-block state ---------------------------------------
            v0f = {}     # flat AND-ed packed view per block
            xv = {}      # bf16 x row views per block
            pT = {}      # xi*xj product tiles
            px = {}      # xcn PSUM pairs
            om = {}      # expansion tiles per (block, mp)
            xij = {}     # xij MLP results
            u_t = {}     # xij hidden

            def prep_block(b):
                ga = [gt[(b, 0)], gt[(b, 1)]]
                # cn_packed = a0 AND a1 (adj byte-chunks 0..9 only, in place)
                v0 = ga[0][:].bitcast(I16)[:, 0:2 * NC5, :]
                v1 = ga[1][:].bitcast(I16)[:, 0:2 * NC5, :]
                nc.vector.tensor_tensor(v0, v0, v1, ALU.bitwise_and)
                v0f[b] = v0.rearrange("p a b -> p (a b)")
                xv[b] = [t[:].bitcast(BF16)[:, 2 * NC5:2 * NC5 + 4, :]
                         .rearrange("p (f s) w -> p f (s w)", f=2) for t in ga]

            def expand(b, mp):
                o = expp.tile([128, 2, NC5, 2 * EB], FP8, tag="exp",
                              name=f"om{b}_{mp}")
                o16 = o[:].bitcast(I16)
                for t in range(2):
                    m = 2 * mp + t
                    dst = o16[:, t, :, :].rearrange("p c e -> p (c e)")
                    if m < 4:
                        nc.vector.tensor_scalar(
                            dst, v0f[b], 4 - m, 0x1010,
                            ALU.logical_shift_left, ALU.bitwise_and)
                    elif m == 4:
                        nc.vector.tensor_scalar(
                            dst, v0f[b], 0x1010, None, ALU.bitwise_and)
                    else:
                        nc.vector.tensor_scalar(
                            dst, v0f[b], m - 4, 0x1010,
                            ALU.logical_shift_right, ALU.bitwise_and)
                om[(b, mp)] = o

            def mm_group(b, mp):
                va = None if (b == 0 and mp == 0) else om[(b, mp)][:]
                for c in range(NC5):
                    base = omc[c][:] if va is None else va[:, :, c, :]
                    for par in range(2):
                        mov = (base
                               .rearrange("p t (i two) -> p t two i", two=2)
                               [:, :, par, :])
                        for fh in range(2):
                            nc.tensor.matmul(
                                px[b][fh][:],
                                x8n_sb[:, mp, c, par, fh, :, :],
                                mov,
                                start=(mp == 0 and c == 0 and par == 0),
                                stop=(mp == 3 and c == NC5 - 1 and par == 1),
                                perf_mode=mybir.MatmulPerfMode.DoubleRow)

            # ---- DVE prep: both blocks' AND/mult/expand queued before any
            # MM-dependent vector work so the MM streams never stall -------
            # block0 mp0 pipelined PER WORD-CHUNK: AND each chunk
            # out-of-place into its own small tile (chunk-granular deps)
            # and expand planes m=0,1 per chunk, so the first 4 matmuls of
            # chunk c can issue ~1us after g2's DMA instead of waiting for
            # the full AND + full-plane expansion (~3us).
            ga0, ga1 = gt[(0, 0)], gt[(0, 1)]
            v0 = ga0[:].bitcast(I16)[:, 0:2 * NC5, :]
            v1 = ga1[:].bitcast(I16)[:, 0:2 * NC5, :]
            omc = []
            for c in range(NC5):
                cc = expp.tile([128, 2, 256], I16, tag="cnc", bufs=NC5,
                               name=f"cnc{c}")
                nc.vector.tensor_tensor(cc[:], v0[:, 2 * c:2 * c + 2, :],
                                        v1[:, 2 * c:2 * c + 2, :],
                                        ALU.bitwise_and)
                oc = expp.tile([128, 2, 2 * EB], FP8, tag="omc", bufs=NC5,
                               name=f"omc{c}")
                oc16 = oc[:].bitcast(I16)
                ccf = cc[:].rearrange("p a b -> p (a b)")
                for t in range(2):   # planes m = 0, 1 -> shift left 4, 3
                    nc.vector.tensor_scalar(
                        oc16[:, t, :], ccf, 4 - t, 0x1010,
                        ALU.logical_shift_left, ALU.bitwise_and)
                omc.append(oc)
            # full AND (in place, original v0 untouched by the above) for
            # the remaining planes' full-tile expansions
            nc.vector.tensor_tensor(v0, v0, v1, ALU.bitwise_and)
            v0f[0] = v0.rearrange("p a b -> p (a b)")
            xv[0] = [t[:].bitcast(BF16)[:, 2 * NC5:2 * NC5 + 4, :]
                     .rearrange("p (f s) w -> p f (s w)", f=2)
                     for t in (ga0, ga1)]
            expand(0, 1)
            # mult AFTER the mp1 planes: pT0 isn't needed until the
            # xij0 layer (~mm group 1), but mp1's planes gate the MM stream
            pT[0] = actp.tile([128, 2, EB], FP32R, tag="act", name="pT0")
            nc.vector.tensor_mul(pT[0][:], xv[0][0], xv[0][1])
            expand(0, 2)
            expand(0, 3)
            prep_block(1)
            pT[1] = actp.tile([128, 2, EB], FP32R, tag="act", name="pT1")
            nc.vector.tensor_mul(pT[1][:], xv[1][0], xv[1][1])
            for mp in range(4):
                expand(1, mp)

            def hcn_layer(b, li, act_eng="mixed"):
                wn, bn, rl = (("w1t", "b1", True), ("w2t", "b2", True),
                              ("w3t", "b3", False))[li]
                src = xcn[b] if li == 0 else h[b]
                h[b] = lin_h(src, wn, bn, rl,
                             actp.tile([128, 2, EB], FP32R, tag="act",
                                       name=f"h{b}_{li}"),
                             act_eng=act_eng,
                             scale=beta_sb if li == 2 else None)

            pox = {}

            def xij_pre_final(b):
                # accumulate xij @ Wl into the final PSUM bank early; the h
                # MMs in combine_final extend the same accumulation group
                pox[b] = pop.tile([1, EB], FP32, tag="po", name=f"pox{b}")
                for k in range(2):
                    nc.tensor.matmul(pox[b][:], wlt_sb[:, k, :],
                                     xij[b][:, k, :], start=(k == 0),
                                     stop=False, skip_group_check=True)

            def combine_final(b):
                # h = h*beta + b3*beta was folded into hcn l3; xij@Wl is
                # already accumulated in pox[b]
                po = pox[b]
                for k in range(2):
                    nc.tensor.matmul(po[:], wlt_sb[:, k, :], h[b][:, k, :],
                                     start=False, stop=(k == 1),
                                     skip_group_check=True)
                nc.scalar.activation(out_sb[:, b * EB:(b + 1) * EB],
                                     po[:], AF.Identity, bias=bl_sb)

            def xcn_copy(b):
                xcn[b] = actp.tile([128, 2, EB], FP32R, tag="act",
                                   name=f"xcn{b}")
                nc.scalar.activation(xcn[b][:, 0, :], px[b][0][:], AF.Copy)
                nc.vector.tensor_scalar(xcn[b][:, 1, :], px[b][1][:],
                                        1.0, 0.0, ALU.mult, ALU.add)

            # ---- Tensor stream: MM0 x80 (xij0 interleaved) --------------
            xcn, h = {}, {}
            px[0] = [pxp.tile([128, EB], FP32, tag="px", name=f"px0_{fh}")
                     for fh in range(2)]
            mm_group(0, 0)
            u_t[0] = lin_h(pT[0], "wat", "ba", True,
                           actp.tile([128, 2, EB], FP32R, tag="act",
                                     name="u0"))
            mm_group(0, 1)
            xij[0] = lin_h(u_t[0], "wbt", "bb", False,
                           actp.tile([128, 2, EB], FP32R, tag="act",
                                     name="xij0"))
            xij_pre_final(0)
            mm_group(0, 2)
            mm_group(0, 3)
            xcn_copy(0)

            # ---- MM1 x80 with xij1 + the whole hcn0 chain interleaved ---
            px[1] = [pxp.tile([128, EB], FP32, tag="px", name=f"px1_{fh}")
                     for fh in range(2)]
            mm_group(1, 0)
            u_t[1] = lin_h(pT[1], "wat", "ba", True,
                           actp.tile([128, 2, EB], FP32R, tag="act",
                                     name="u1"))
            hcn_layer(0, 0)
            mm_group(1, 1)
            xij[1] = lin_h(u_t[1], "wbt", "bb", False,
                           actp.tile([128, 2, EB], FP32R, tag="act",
                                     name="xij1"))
            xij_pre_final(1)
            hcn_layer(0, 1)
            mm_group(1, 2)
            hcn_layer(0, 2)
            mm_group(1, 3)
            combine_final(0)
            nc.sync.dma_start(out_d[:, 0:EB], out_sb[:, 0:EB])
            xcn_copy(1)

            # ---- tail: only hcn1 remains ------------------------------
            for li in range(3):
                hcn_layer(1, li)
            combine_final(1)

            nc.sync.dma_start(out_d[:, EB:EC], out_sb[:, EB:EC])

    nc.compile()
    return nc


def _wrap_idx(ids, num):
    """Pack indices for dma_gather: [128, num//16] int16, idx i at
    [i % 16, i // 16], replicated over the 8 groups of 16 partitions."""
    a = np.asarray(ids).astype(np.int16)
    w = a.reshape(num // 16, 16).T.copy()
    return np.ascontiguousarray(np.tile(w, (8, 1)))


def prepare_inputs(x, adj, edge, W1, b1, W2, b2, W3, b3, Wa, ba, Wb, bb,
                   Wl, bl, beta):
    x = np.asarray(x, np.float32)
    adj = np.asarray(adj, np.float32)
    edge = np.asarray(edge)

    # extended rows: [packed adj bits (1280B) | x bf16 (512B)]
    adjp = np.zeros((N, NPAD), np.uint8)
    adjp[:, :N] = (adj != 0)
    adjx8 = np.zeros((N, ROWB), np.uint8)
    adjx8[:, :PKB] = np.packbits(adjp, axis=1, bitorder="little")
    adjx8[:, PKB:] = np.ascontiguousarray(
        x.astype(BF16_NP)).view(np.uint8).reshape(N, 2 * D)
    adjx = adjx8.view(FP8_NP)

    # permuted + scaled fp8 x table, mp-major:
    # x8n[p, mp, c, par, fh, t, f] = 32*x[2048c + 16p + 8par + 2mp + t,
    #                                     fh*128 + f]
    x8 = np.zeros((NPAD, D), FP8_NP)
    x8[:N] = np.clip(x * XSCALE, -224.0, 224.0).astype(FP8_NP)
    p_, mp_, c_, par_, t_ = np.meshgrid(
        np.arange(128), np.arange(4), np.arange(NC5), np.arange(2),
        np.arange(2), indexing="ij")
    nodes = 2048 * c_ + 16 * p_ + 8 * par_ + 2 * mp_ + t_
    tbl = x8[nodes]                                    # [128,4,5,2,2,256]
    tbl = tbl.reshape(128, 4, NC5, 2, 2, 2, 128)       # t, fh, f
    tbl = np.ascontiguousarray(tbl.transpose(0, 1, 2, 3, 5, 4, 6))
    x8n = tbl.reshape(128, -1)

    # packed weights: per partition p (fp32 elems):
    #   [5 x (k,h)=2x256 transposed weights][wlt 2][5 x bias 2][bl][beta][pad]
    wpack = np.zeros((128, 2576), np.float32)
    for i, W in enumerate((Wa, Wb, W1, W2, W3)):
        wt = np.asarray(W, np.float32).T.reshape(2, 128, H)   # [k, p, h]
        wpack[:, i * 512:(i + 1) * 512] = wt.transpose(1, 0, 2).reshape(128, 512)
    wpack[:, 2560:2562] = np.asarray(Wl, np.float32).T.reshape(2, 128).T
    beta0 = np.asarray(beta, np.float32).reshape(-1)[0]
    b3beta = np.asarray(b3, np.float32) * beta0   # *beta folded into hcn l3
    for i, bv in enumerate((ba, bb, b1, b2, b3beta)):
        wpack[:, 2562 + 2 * i:2564 + 2 * i] = \
            np.asarray(bv, np.float32).reshape(2, 128).T
    wpack[:, 2572] = np.asarray(bl, np.float32).reshape(-1)[0]
    wpack[:, 2573] = np.asarray(beta, np.float32).reshape(-1)[0]

    common = dict(adjx=adjx, x8n=x8n, wpack=wpack)
    in_maps = []
    for c in range(N_CORES):
        m = dict(common)
        gi = []
        for b in range(NB):
            sl = slice(c * EC + b * EB, c * EC + (b + 1) * EB)
            for s in range(2):
                gi.append(_wrap_idx(edge[sl, s], EB))
        m["idxg"] = np.ascontiguousarray(np.hstack(gi))
        in_maps.append(m)
    return in_maps


_CACHE = {}


def _get_program():
    if "nc" not in _CACHE:
        _CACHE["nc"] = build_program()
    return _CACHE["nc"]


def run(in_maps, **kw):
    nc = _get_program()
    return run_bass_kernel_spmd(nc, in_maps, list(range(N_CORES)), **kw)


def kernel(**inputs):
    in_maps = prepare_inputs(**inputs)
    res = run(in_maps)
    out = np.concatenate([res.results[c]["out"][0] for c in range(N_CORES)])
    return out.reshape(E, 1).astype(np.float32)


# revision 46
# speedup vs baseline: 1.0145x; 1.0145x over previous
"""CNLP (common-neighbor link prediction) kernel for Trainium2, 8 NeuronCores.

Reference computation (per query edge e = (i, j)):
    cn  = adj[i] * adj[j]                      # common-neighbor indicator [N]
    xcn = cn @ x                               # sum of common-neighbor feats
    xij = relu(x[i]*x[j] @ Wa.T + ba) @ Wb.T + bb
    hcn = (relu->relu->lin) 3-layer MLP on xcn
    out = (hcn * beta + xij) @ Wl.T + bl       # [E, 1]

Sharding: edges (E=8192) split 8 x 1024 across cores; adj/x/weights replicated.

Device strategy per core (1024 edges in 2 blocks of 512):
  - adj is binary -> BIT-PACKED host-side (10240 nodes -> 1280 bytes/row).
    Extended row [packed 1280B | x bf16 512B]; one gpsimd
    dma_gather(transpose=True) per (block, src).  The q7 library for the
    gather is preloaded with load_library(mlp) as the first pool
    instruction so its ~14us load overlaps the input DMAs.
  - DVE ANDs the packed pair (u16 2x mode) then EXPANDS bits to fp8 bytes
    with 8 fused shift+mask tensor_scalar ops per block:
        OUT[p, m, c, e] = shift_m(cn_packed[p, c, e]) & 0x1010
    giving fp8 byte 0x10 (=2^-5) at node 2048c + 16p + 8par + m (par = byte
    within the u16 lane).  The node permutation is absorbed into the
    host-permuted stationary x table (pre-scaled by 32: 2^-5 * 32x = x).
  - Big matmul FLIPPED: stationary = permuted fp8 x table, moving = expanded
    cn slices; PSUM accumulates xcn^T feature-major [128f, 512e].  DoubleRow
    fp8 perf mode (2 k-tiles = adjacent m-planes) for 2x PE rate.
  - Both blocks' big-MM streams run back-to-back (block1's AND/expand are
    queued on DVE before any MLP vector work); the xij 2-layer MLPs are
    interleaved INTO the MM streams; the two hcn tail chains run
    interleaved on separate act engines (scalar for b0, vector for b1).
"""

import numpy as np
import ml_dtypes

import concourse.bacc as bacc
import concourse.tile as tile
import concourse.mybir as mybir
from concourse import library_config
from concourse.bass_utils import run_bass_kernel_spmd

BF16 = mybir.dt.bfloat16
FP32 = mybir.dt.float32
FP32R = mybir.dt.float32r
FP8 = mybir.dt.float8e4
I16 = mybir.dt.int16
AF = mybir.ActivationFunctionType
ALU = mybir.AluOpType
BF16_NP = ml_dtypes.bfloat16
FP8_NP = ml_dtypes.float8_e4m3

N_CORES = 8
N, E, D, H = 10000, 8192, 256, 256
NPAD = 10240                      # n padded to a multiple of 2048
EC = E // N_CORES                 # 1024 edges per core
EB = 512                          # edges per block
NB = EC // EB                     # 2 blocks
PKB = NPAD // 8                   # 1280 packed adjacency bytes per row
ROWB = PKB + 2 * D                # 1792 bytes per extended row
NC5 = PKB // 256                  # 5 u16 word-chunks of packed bits
XSCALE = 32.0                     # x table pre-scale (cn byte is 2^-5)


def build_program():
    nc = bacc.Bacc("TRN2", target_bir_lowering=False, debug=False,
                   enable_asserts=False, num_devices=N_CORES)

    adjx = nc.dram_tensor("adjx", [N, ROWB], FP8, kind="ExternalInput")
    # permuted+scaled fp8 x table, mp-major: [p][mp][c][par][fh][t][f]
    x8n_d = nc.dram_tensor("x8n", [128, 4 * NC5 * 2 * 2 * 2 * 128], FP8,
                           kind="ExternalInput")
    idxg_d = nc.dram_tensor("idxg", [128, NB * 2 * EB // 16], I16,
                            kind="ExternalInput")
    wpack_d = nc.dram_tensor("wpack", [128, 2576], FP32, kind="ExternalInput")
    out_d = nc.dram_tensor("out", [1, EC], FP32, kind="ExternalOutput")

    with tile.TileContext(nc) as tc:
        # library load FIRST (before pool-init memsets): its ~13us q7 code
        # DMA gates the first gather and overlaps the input loads
        nc.gpsimd.load_library(library_config.mlp)
        with (
            tc.tile_pool(name="const", bufs=1) as constp,
            tc.tile_pool(name="gath", bufs=4) as gathp,
            tc.tile_pool(name="exp", bufs=8) as expp,
            tc.tile_pool(name="acts", bufs=10) as actp,
            tc.tile_pool(name="px", bufs=4, space="PSUM") as pxp,
            tc.tile_pool(name="pm", bufs=2, space="PSUM") as pmp,
            tc.tile_pool(name="po", bufs=2, space="PSUM") as pop,
        ):
            idxg_sb = constp.tile([128, NB * 2 * EB // 16], I16)
            nc.sync.dma_start(idxg_sb[:], idxg_d[:])

            # all 4 gathers upfront (pool queue; ~5us decode each, serial)
            gt = {}
            for b in range(NB):
                for s in range(2):
                    gsl = slice((2 * b + s) * EB // 16,
                                (2 * b + s + 1) * EB // 16)
                    t = gathp.tile([128, ROWB // 128, EB], FP8, tag="g",
                                   bufs=4, name=f"a{b}{s}")
                    nc.gpsimd.dma_gather(
                        t[:], adjx[:], idxg_sb[:, gsl], EB, EB,
                        elem_size=ROWB, transpose=True)
                    gt[(b, s)] = t

            # fp8 x table, split by mp quarter so early matmuls start sooner
            x8n_sb = constp.tile([128, 4, NC5, 2, 2, 2, 128], FP8)
            qsz = NC5 * 2 * 2 * 2 * 128
            for mp in range(4):
                nc.sync.dma_start(
                    x8n_sb[:, mp, :, :, :, :, :]
                    .rearrange("p c q f t g -> p (c q f t g)"),
                    x8n_d[:, mp * qsz:(mp + 1) * qsz])

            # weights: one packed DMA -> scalar-copy to fp32r (the BIR
            # verifier requires fp32r matmul operands to be fp32r-rounded)
            wpack = constp.tile([128, 2576], FP32)
            nc.sync.dma_start(wpack[:], wpack_d[:])
            w_sb = {}
            for i, nm in enumerate(("wat", "wbt", "w1t", "w2t", "w3t")):
                t = constp.tile([128, 2, H], FP32R, tag=f"w_{nm}")
                nc.scalar.activation(
                    t[:], wpack[:, i * 512:(i + 1) * 512]
                    .rearrange("p (k h) -> p k h", k=2), AF.Copy)
                w_sb[nm] = t
            wlt_sb = constp.tile([128, 2, 1], FP32R)
            nc.scalar.activation(
                wlt_sb[:], wpack[:, 2560:2562]
                .rearrange("p (k o) -> p k o", k=2), AF.Copy)
            b_sb = {}
            for i, nm in enumerate(("ba", "bb", "b1", "b2", "b3")):
                b_sb[nm] = (wpack[:, 2562 + 2 * i:2564 + 2 * i]
                            .rearrange("p (k o) -> p k o", k=2))
            bl_sb = wpack[0:1, 2572:2573]
            beta_sb = wpack[:, 2573:2574]

            out_sb = constp.tile([1, EC], FP32)

            # DVE warm-up: the first instruction of each ALU-op combo pays a
            # ~1.5-3us uop-program load; DVE is idle during the ~35us gather
            # startup, so prime every combo used later on a scrap tile.
            scrap = constp.tile([128, 32], I16)
            scrap2 = constp.tile([128, 32], FP32)
            nc.vector.tensor_tensor(scrap[:, 0:16], scrap[:, 0:16],
                                    scrap[:, 16:32], ALU.bitwise_and)
            nc.vector.tensor_scalar(scrap[:, 0:16], scrap[:, 16:32], 2,
                                    0x1010, ALU.logical_shift_left,
                                    ALU.bitwise_and)
            nc.vector.tensor_scalar(scrap[:, 0:16], scrap[:, 16:32], 2,
                                    0x1010, ALU.logical_shift_right,
                                    ALU.bitwise_and)
            nc.vector.tensor_scalar(scrap[:, 0:16], scrap[:, 16:32],
                                    0x1010, None, ALU.bitwise_and)
            nc.vector.tensor_scalar(scrap2[:, 0:16], scrap2[:, 16:32],
                                    1.0, 0.0, ALU.mult, ALU.add)
            nc.vector.tensor_scalar(scrap2[:, 0:16], scrap2[:, 16:32],
                                    0.0, 0.0, ALU.add, ALU.max)
            nc.vector.tensor_tensor(scrap2[:, 0:16], scrap2[:, 0:16],
                                    scrap2[:, 16:32], ALU.add)
            nc.vector.tensor_tensor(scrap2[:, 0:16], scrap2[:, 0:16],
                                    scrap2[:, 16:32], ALU.mult)

            # MLP layer, feature-major fp32r (fp22 reads), 512 edges.
            # act_eng: 'scalar' | 'vector' | 'mixed' (t=0 scalar, t=1 vector
            # so the two half-activations run concurrently)
            # scale: optional AP multiplied into the pre-bias value (used to
            # fold *beta into the last hcn layer; its bias is b3*beta)
            def lin_h(src, wname, bname, relu, dst, act_eng="scalar",
                      scale=None):
                w, bias = w_sb[wname], b_sb[bname]
                for t in range(2):
                    pm = pmp.tile([128, EB], FP32, tag="pm")
                    for k in range(2):
                        nc.tensor.matmul(
                            pm[:], w[:, k, t * 128:(t + 1) * 128],
                            src[:, k, :], start=(k == 0), stop=(k == 1))
                    dsl = dst[:, t, :]
                    use_scalar = (act_eng == "scalar"
                                  or (act_eng == "mixed" and t == 0))
                    if use_scalar:
                        nc.scalar.activation(
                            dsl, pm[:], AF.Relu if relu else AF.Identity,
                            bias=bias[:, t, :],
                            scale=scale if scale is not None else 1.0)
                    elif scale is not None:
                        nc.vector.tensor_scalar(
                            dsl, pm[:], scale, bias[:, t, :],
                            ALU.mult, ALU.add)
                    elif relu:
                        nc.vector.tensor_scalar(
                            dsl, pm[:], bias[:, t, :], 0.0,
                            ALU.add, ALU.max)
                    else:
                        nc.vector.tensor_scalar_add(dsl, pm[:], bias[:, t, :])
                return dst

            # ---- per-block state ---------------------------------------
            v0f = {}     # flat AND-ed packed view per block
            xv = {}      # bf16 x row views per block
            pT = {}      # xi*xj product tiles
            px = {}      # xcn PSUM pairs
            om = {}      # expansion tiles per (block, mp)
            xij = {}     # xij MLP results
            u_t = {}     # xij hidden

            def prep_block(b):
                ga = [gt[(b, 0)], gt[(b, 1)]]
                # cn_packed = a0 AND a1 (adj byte-chunks 0..9 only, in place)
                v0 = ga[0][:].bitcast(I16)[:, 0:2 * NC5, :]
                v1 = ga[1][:].bitcast(I16)[:, 0:2 * NC5, :]
                nc.vector.tensor_tensor(v0, v0, v1, ALU.bitwise_and)
                v0f[b] = v0.rearrange("p a b -> p (a b)")
                xv[b] = [t[:].bitcast(BF16)[:, 2 * NC5:2 * NC5 + 4, :]
                         .rearrange("p (f s) w -> p f (s w)", f=2) for t in ga]

            def expand(b, mp):
                o = expp.tile([128, 2, NC5, 2 * EB], FP8, tag="exp",
                              name=f"om{b}_{mp}")
                o16 = o[:].bitcast(I16)
                for t in range(2):
                    m = 2 * mp + t
                    dst = o16[:, t, :, :].rearrange("p c e -> p (c e)")
                    if m < 4:
                        nc.vector.tensor_scalar(
                            dst, v0f[b], 4 - m, 0x1010,
                            ALU.logical_shift_left, ALU.bitwise_and)
                    elif m == 4:
                        nc.vector.tensor_scalar(
                            dst, v0f[b], 0x1010, None, ALU.bitwise_and)
                    else:
                        nc.vector.tensor_scalar(
                            dst, v0f[b], m - 4, 0x1010,
                            ALU.logical_shift_right, ALU.bitwise_and)
                om[(b, mp)] = o

            def mm_group(b, mp):
                va = None if (b == 0 and mp == 0) else om[(b, mp)][:]
                for c in range(NC5):
                    base = omc[c][:] if va is None else va[:, :, c, :]
                    for par in range(2):
                        mov = (base
                               .rearrange("p t (i two) -> p t two i", two=2)
                               [:, :, par, :])
                        for fh in range(2):
                            nc.tensor.matmul(
                                px[b][fh][:],
                                x8n_sb[:, mp, c, par, fh, :, :],
                                mov,
                                start=(mp == 0 and c == 0 and par == 0),
                                stop=(mp == 3 and c == NC5 - 1 and par == 1),
                                perf_mode=mybir.MatmulPerfMode.DoubleRow)

            # ---- DVE prep: both blocks' AND/mult/expand queued before any
            # MM-dependent vector work so the MM streams never stall -------
            # block0 mp0 pipelined PER WORD-CHUNK: AND each chunk
            # out-of-place into its own small tile (chunk-granular deps)
            # and expand planes m=0,1 per chunk, so the first 4 matmuls of
            # chunk c can issue ~1us after g2's DMA instead of waiting for
            # the full AND + full-plane expansion (~3us).
            ga0, ga1 = gt[(0, 0)], gt[(0, 1)]
            v0 = ga0[:].bitcast(I16)[:, 0:2 * NC5, :]
            v1 = ga1[:].bitcast(I16)[:, 0:2 * NC5, :]
            omc = []
            for c in range(NC5):
                cc = expp.tile([128, 2, 256], I16, tag="cnc", bufs=NC5,
                               name=f"cnc{c}")
                nc.vector.tensor_tensor(cc[:], v0[:, 2 * c:2 * c + 2, :],
                                        v1[:, 2 * c:2 * c + 2, :],
                                        ALU.bitwise_and)
                oc = expp.tile([128, 2, 2 * EB], FP8, tag="omc", bufs=NC5,
                               name=f"omc{c}")
                oc16 = oc[:].bitcast(I16)
                ccf = cc[:].rearrange("p a b -> p (a b)")
                for t in range(2):   # planes m = 0, 1 -> shift left 4, 3
                    nc.vector.tensor_scalar(
                        oc16[:, t, :], ccf, 4 - t, 0x1010,
                        ALU.logical_shift_left, ALU.bitwise_and)
                omc.append(oc)
            # full AND (in place, original v0 untouched by the above) for
            # the remaining planes' full-tile expansions
            nc.vector.tensor_tensor(v0, v0, v1, ALU.bitwise_and)
            v0f[0] = v0.rearrange("p a b -> p (a b)")
            xv[0] = [t[:].bitcast(BF16)[:, 2 * NC5:2 * NC5 + 4, :]
                     .rearrange("p (f s) w -> p f (s w)", f=2)
                     for t in (ga0, ga1)]
            expand(0, 1)
            # mult AFTER the mp1 planes: pT0 isn't needed until the
            # xij0 layer (~mm group 1), but mp1's planes gate the MM stream
            pT[0] = actp.tile([128, 2, EB], FP32R, tag="act", name="pT0")
            nc.vector.tensor_mul(pT[0][:], xv[0][0], xv[0][1])
            expand(0, 2)
            expand(0, 3)
            prep_block(1)
            pT[1] = actp.tile([128, 2, EB], FP32R, tag="act", name="pT1")
            nc.vector.tensor_mul(pT[1][:], xv[1][0], xv[1][1])
            for mp in range(4):
                expand(1, mp)

            def hcn_layer(b, li, act_eng="mixed"):
                wn, bn, rl = (("w1t", "b1", True), ("w2t", "b2", True),
                              ("w3t", "b3", False))[li]
                src = xcn[b] if li == 0 else h[b]
                h[b] = lin_h(src, wn, bn, rl,
                             actp.tile([128, 2, EB], FP32R, tag="act",
                                       name=f"h{b}_{li}"),
                             act_eng=act_eng,
                             scale=beta_sb if li == 2 else None)

            pox = {}

            def xij_pre_final(b):
                # accumulate xij @ Wl into the final PSUM bank early; the h
                # MMs in combine_final extend the same accumulation group
                pox[b] = pop.tile([1, EB], FP32, tag="po", name=f"pox{b}")
                for k in range(2):
                    nc.tensor.matmul(pox[b][:], wlt_sb[:, k, :],
                                     xij[b][:, k, :], start=(k == 0),
                                     stop=False, skip_group_check=True)

            def combine_final(b):
                # h = h*beta + b3*beta was folded into hcn l3; xij@Wl is
                # already accumulated in pox[b]
                po = pox[b]
                for k in range(2):
                    nc.tensor.matmul(po[:], wlt_sb[:, k, :], h[b][:, k, :],
                                     start=False, stop=(k == 1),
                                     skip_group_check=True)
                nc.scalar.activation(out_sb[:, b * EB:(b + 1) * EB],
                                     po[:], AF.Identity, bias=bl_sb)

            def xcn_copy(b):
                xcn[b] = actp.tile([128, 2, EB], FP32R, tag="act",
                                   name=f"xcn{b}")
                nc.scalar.activation(xcn[b][:, 0, :], px[b][0][:], AF.Copy)
                nc.vector.tensor_scalar(xcn[b][:, 1, :], px[b][1][:],
                                        1.0, 0.0, ALU.mult, ALU.add)

            # ---- Tensor stream: MM0 x80 (xij0 interleaved) --------------
            xcn, h = {}, {}
            px[0] = [pxp.tile([128, EB], FP32, tag="px", name=f"px0_{fh}")
                     for fh in range(2)]
            mm_group(0, 0)
            u_t[0] = lin_h(pT[0], "wat", "ba", True,
                           actp.tile([128, 2, EB], FP32R, tag="act",
                                     name="u0"))
            mm_group(0, 1)
            xij[0] = lin_h(u_t[0], "wbt", "bb", False,
                           actp.tile([128, 2, EB], FP32R, tag="act",
                                     name="xij0"))
            xij_pre_final(0)
            mm_group(0, 2)
            mm_group(0, 3)
            xcn_copy(0)

            # ---- MM1 x80 with xij1 + the whole hcn0 chain interleaved ---
            px[1] = [pxp.tile([128, EB], FP32, tag="px", name=f"px1_{fh}")
                     for fh in range(2)]
            mm_group(1, 0)
            u_t[1] = lin_h(pT[1], "wat", "ba", True,
                           actp.tile([128, 2, EB], FP32R, tag="act",
                                     name="u1"))
            hcn_layer(0, 0)
            mm_group(1, 1)
            xij[1] = lin_h(u_t[1], "wbt", "bb", False,
                           actp.tile([128, 2, EB], FP32R, tag="act",
                                     name="xij1"))
            xij_pre_final(1)
            hcn_layer(0, 1)
            mm_group(1, 2)
            hcn_layer(0, 2)
            mm_group(1, 3)
            combine_final(0)
            nc.sync.dma_start(out_d[:, 0:EB], out_sb[:, 0:EB])
            xcn_copy(1)

            # ---- tail: only hcn1 remains ------------------------------
            for li in range(3):
                hcn_layer(1, li)
            combine_final(1)

            nc.sync.dma_start(out_d[:, EB:EC], out_sb[:, EB:EC])

    nc.compile()
    return nc


def _wrap_idx(ids, num):
    """Pack indices for dma_gather: [128, num//16] int16, idx i at
    [i % 16, i // 16], replicated over the 8 groups of 16 partitions."""
    a = np.asarray(ids).astype(np.int16)
    w = a.reshape(num // 16, 16).T.copy()
    return np.ascontiguousarray(np.tile(w, (8, 1)))


def prepare_inputs(x, adj, edge, W1, b1, W2, b2, W3, b3, Wa, ba, Wb, bb,
                   Wl, bl, beta):
    x = np.asarray(x, np.float32)
    adj = np.asarray(adj, np.float32)
    edge = np.asarray(edge)

    # extended rows: [packed adj bits (1280B) | x bf16 (512B)]
    adjp = np.zeros((N, NPAD), np.uint8)
    adjp[:, :N] = (adj != 0)
    adjx8 = np.zeros((N, ROWB), np.uint8)
    adjx8[:, :PKB] = np.packbits(adjp, axis=1, bitorder="little")
    adjx8[:, PKB:] = np.ascontiguousarray(
        x.astype(BF16_NP)).view(np.uint8).reshape(N, 2 * D)
    adjx = adjx8.view(FP8_NP)

    # permuted + scaled fp8 x table, mp-major:
    # x8n[p, mp, c, par, fh, t, f] = 32*x[2048c + 16p + 8par + 2mp + t,
    #                                     fh*128 + f]
    x8 = np.zeros((NPAD, D), FP8_NP)
    x8[:N] = np.clip(x * XSCALE, -224.0, 224.0).astype(FP8_NP)
    p_, mp_, c_, par_, t_ = np.meshgrid(
        np.arange(128), np.arange(4), np.arange(NC5), np.arange(2),
        np.arange(2), indexing="ij")
    nodes = 2048 * c_ + 16 * p_ + 8 * par_ + 2 * mp_ + t_
    tbl = x8[nodes]                                    # [128,4,5,2,2,256]
    tbl = tbl.reshape(128, 4, NC5, 2, 2, 2, 128)       # t, fh, f
    tbl = np.ascontiguousarray(tbl.transpose(0, 1, 2, 3, 5, 4, 6))
    x8n = tbl.reshape(128, -1)

    # packed weights: per partition p (fp32 elems):
    #   [5 x (k,h)=2x256 transposed weights][wlt 2][5 x bias 2][bl][beta][pad]
    wpack = np.zeros((128, 2576), np.float32)
    for i, W in enumerate((Wa, Wb, W1, W2, W3)):
        wt = np.asarray(W, np.float32).T.reshape(2, 128, H)   # [k, p, h]
        wpack[:, i * 512:(i + 1) * 512] = wt.transpose(1, 0, 2).reshape(128, 512)
    wpack[:, 2560:2562] = np.asarray(Wl, np.float32).T.reshape(2, 128).T
    beta0 = np.asarray(beta, np.float32).reshape(-1)[0]
    b3beta = np.asarray(b3, np.float32) * beta0   # *beta folded into hcn l3
    for i, bv in enumerate((ba, bb, b1, b2, b3beta)):
        wpack[:, 2562 + 2 * i:2564 + 2 * i] = \
            np.asarray(bv, np.float32).reshape(2, 128).T
    wpack[:, 2572] = np.asarray(bl, np.float32).reshape(-1)[0]
    wpack[:, 2573] = np.asarray(beta, np.float32).reshape(-1)[0]

    common = dict(adjx=adjx, x8n=x8n, wpack=wpack)
    in_maps = []
    for c in range(N_CORES):
        m = dict(common)
        gi = []
        for b in range(NB):
            sl = slice(c * EC + b * EB, c * EC + (b + 1) * EB)
            for s in range(2):
                gi.append(_wrap_idx(edge[sl, s], EB))
        m["idxg"] = np.ascontiguousarray(np.hstack(gi))
        in_maps.append(m)
    return in_maps


_CACHE = {}


def _get_program():
    if "nc" not in _CACHE:
        _CACHE["nc"] = build_program()
    return _CACHE["nc"]


def run(in_maps, **kw):
    nc = _get_program()
    return run_bass_kernel_spmd(nc, in_maps, list(range(N_CORES)), **kw)


def kernel(**inputs):
    in_maps = prepare_inputs(**inputs)
    res = run(in_maps)
    out = np.concatenate([res.results[c]["out"][0] for c in range(N_CORES)])
    return out.reshape(E, 1).astype(np.float32)


# revision 48
# speedup vs baseline: 1.0446x; 1.0297x over previous
"""CNLP (common-neighbor link prediction) kernel for Trainium2, 8 NeuronCores.

Reference computation (per query edge e = (i, j)):
    cn  = adj[i] * adj[j]                      # common-neighbor indicator [N]
    xcn = cn @ x                               # sum of common-neighbor feats
    xij = relu(x[i]*x[j] @ Wa.T + ba) @ Wb.T + bb
    hcn = (relu->relu->lin) 3-layer MLP on xcn
    out = (hcn * beta + xij) @ Wl.T + bl       # [E, 1]

Sharding: edges (E=8192) split 8 x 1024 across cores; adj/x/weights replicated.

Device strategy per core (1024 edges in 2 blocks of 512):
  - adj is binary -> BIT-PACKED host-side (10240 nodes -> 1280 bytes/row).
    Extended row [packed 1280B | x bf16 512B]; one gpsimd
    dma_gather(transpose=True) per (block, src).  The q7 library for the
    gather is preloaded with load_library(mlp) as the first pool
    instruction so its ~14us load overlaps the input DMAs.
  - DVE ANDs the packed pair (u16 2x mode) then EXPANDS bits to fp8 bytes
    with 8 fused shift+mask tensor_scalar ops per block:
        OUT[p, m, c, e] = shift_m(cn_packed[p, c, e]) & 0x1010
    giving fp8 byte 0x10 (=2^-5) at node 2048c + 16p + 8par + m (par = byte
    within the u16 lane).  The node permutation is absorbed into the
    host-permuted stationary x table (pre-scaled by 32: 2^-5 * 32x = x).
  - Big matmul FLIPPED: stationary = permuted fp8 x table, moving = expanded
    cn slices; PSUM accumulates xcn^T feature-major [128f, 512e].  DoubleRow
    fp8 perf mode (2 k-tiles = adjacent m-planes) for 2x PE rate.
  - Both blocks' big-MM streams run back-to-back (block1's AND/expand are
    queued on DVE before any MLP vector work); the xij 2-layer MLPs are
    interleaved INTO the MM streams; the two hcn tail chains run
    interleaved on separate act engines (scalar for b0, vector for b1).
"""

import numpy as np
import ml_dtypes

import concourse.bacc as bacc
import concourse.tile as tile
import concourse.mybir as mybir
from concourse import library_config
from concourse.bass_utils import run_bass_kernel_spmd

BF16 = mybir.dt.bfloat16
FP32 = mybir.dt.float32
FP32R = mybir.dt.float32r
FP8 = mybir.dt.float8e4
I16 = mybir.dt.int16
AF = mybir.ActivationFunctionType
ALU = mybir.AluOpType
BF16_NP = ml_dtypes.bfloat16
FP8_NP = ml_dtypes.float8_e4m3

N_CORES = 8
N, E, D, H = 10000, 8192, 256, 256
NPAD = 10240                      # n padded to a multiple of 2048
EC = E // N_CORES                 # 1024 edges per core
EB = 512                          # edges per block
NB = EC // EB                     # 2 blocks
PKB = NPAD // 8                   # 1280 packed adjacency bytes per row
ROWB = PKB + 2 * D                # 1792 bytes per extended row
NC5 = PKB // 256                  # 5 u16 word-chunks of packed bits
XSCALE = 32.0                     # x table pre-scale (cn byte is 2^-5)


def build_program():
    nc = bacc.Bacc("TRN2", target_bir_lowering=False, debug=False,
                   enable_asserts=False, num_devices=N_CORES)

    adjx = nc.dram_tensor("adjx", [N, ROWB], FP8, kind="ExternalInput")
    # permuted+scaled fp8 x table, mp-major: [p][mp][c][par][fh][t][f]
    x8n_d = nc.dram_tensor("x8n", [128, 4 * NC5 * 2 * 2 * 2 * 128], FP8,
                           kind="ExternalInput")
    idxg_d = nc.dram_tensor("idxg", [128, NB * 2 * EB // 16], I16,
                            kind="ExternalInput")
    wpack_d = nc.dram_tensor("wpack", [128, 2576], FP32, kind="ExternalInput")
    out_d = nc.dram_tensor("out", [1, EC], FP32, kind="ExternalOutput")

    with tile.TileContext(nc) as tc:
        # library load FIRST (before pool-init memsets): its ~13us q7 code
        # DMA gates the first gather and overlaps the input loads
        nc.gpsimd.load_library(library_config.mlp)
        with (
            tc.tile_pool(name="const", bufs=1) as constp,
            tc.tile_pool(name="gath", bufs=4) as gathp,
            tc.tile_pool(name="exp", bufs=8) as expp,
            tc.tile_pool(name="acts", bufs=10) as actp,
            tc.tile_pool(name="px", bufs=4, space="PSUM") as pxp,
            tc.tile_pool(name="pm", bufs=2, space="PSUM") as pmp,
            tc.tile_pool(name="po", bufs=2, space="PSUM") as pop,
        ):
            idxg_sb = constp.tile([128, NB * 2 * EB // 16], I16)
            nc.sync.dma_start(idxg_sb[:], idxg_d[:])

            # all 4 gathers upfront (pool queue; ~5us decode each, serial)
            gt = {}
            for b in range(NB):
                for s in range(2):
                    gsl = slice((2 * b + s) * EB // 16,
                                (2 * b + s + 1) * EB // 16)
                    t = gathp.tile([128, ROWB // 128, EB], FP8, tag="g",
                                   bufs=4, name=f"a{b}{s}")
                    nc.gpsimd.dma_gather(
                        t[:], adjx[:], idxg_sb[:, gsl], EB, EB,
                        elem_size=ROWB, transpose=True)
                    gt[(b, s)] = t

            # fp8 x table, split by mp quarter so early matmuls start sooner
            x8n_sb = constp.tile([128, 4, NC5, 2, 2, 2, 128], FP8)
            qsz = NC5 * 2 * 2 * 2 * 128
            for mp in range(4):
                nc.sync.dma_start(
                    x8n_sb[:, mp, :, :, :, :, :]
                    .rearrange("p c q f t g -> p (c q f t g)"),
                    x8n_d[:, mp * qsz:(mp + 1) * qsz])

            # weights: one packed DMA -> scalar-copy to fp32r (the BIR
            # verifier requires fp32r matmul operands to be fp32r-rounded)
            wpack = constp.tile([128, 2576], FP32)
            nc.sync.dma_start(wpack[:], wpack_d[:])
            w_sb = {}
            for i, nm in enumerate(("wat", "wbt", "w1t", "w2t")):
                t = constp.tile([128, 2, H], FP32R, tag=f"w_{nm}")
                nc.scalar.activation(
                    t[:], wpack[:, i * 512:(i + 1) * 512]
                    .rearrange("p (k h) -> p k h", k=2), AF.Copy)
                w_sb[nm] = t
            wlt_sb = constp.tile([128, 2, 1], FP32R)
            nc.scalar.activation(
                wlt_sb[:], wpack[:, 2560:2562]
                .rearrange("p (k o) -> p k o", k=2), AF.Copy)
            # folded final weights: hcn layer 3 is LINEAR, so
            # beta*(Wl @ W3) replaces it entirely (host-folded, exact)
            wfold_sb = constp.tile([128, 2, 1], FP32R)
            nc.scalar.activation(
                wfold_sb[:], wpack[:, 2570:2572]
                .rearrange("p (k o) -> p k o", k=2), AF.Copy)
            b_sb = {}
            for i, nm in enumerate(("ba", "bb", "b1", "b2")):
                b_sb[nm] = (wpack[:, 2562 + 2 * i:2564 + 2 * i]
                            .rearrange("p (k o) -> p k o", k=2))
            bl_sb = wpack[0:1, 2572:2573]
            beta_sb = wpack[:, 2573:2574]

            out_sb = constp.tile([1, EC], FP32)

            # DVE warm-up: the first instruction of each ALU-op combo pays a
            # ~1.5-3us uop-program load; DVE is idle during the ~35us gather
            # startup, so prime every combo used later on a scrap tile.
            scrap = constp.tile([128, 32], I16)
            scrap2 = constp.tile([128, 32], FP32)
            nc.vector.tensor_tensor(scrap[:, 0:16], scrap[:, 0:16],
                                    scrap[:, 16:32], ALU.bitwise_and)
            nc.vector.tensor_scalar(scrap[:, 0:16], scrap[:, 16:32], 2,
                                    0x1010, ALU.logical_shift_left,
                                    ALU.bitwise_and)
            nc.vector.tensor_scalar(scrap[:, 0:16], scrap[:, 16:32], 2,
                                    0x1010, ALU.logical_shift_right,
                                    ALU.bitwise_and)
            nc.vector.tensor_scalar(scrap[:, 0:16], scrap[:, 16:32],
                                    0x1010, None, ALU.bitwise_and)
            nc.vector.tensor_scalar(scrap2[:, 0:16], scrap2[:, 16:32],
                                    1.0, 0.0, ALU.mult, ALU.add)
            nc.vector.tensor_scalar(scrap2[:, 0:16], scrap2[:, 16:32],
                                    0.0, 0.0, ALU.add, ALU.max)
            nc.vector.tensor_tensor(scrap2[:, 0:16], scrap2[:, 0:16],
                                    scrap2[:, 16:32], ALU.add)
            nc.vector.tensor_tensor(scrap2[:, 0:16], scrap2[:, 0:16],
                                    scrap2[:, 16:32], ALU.mult)

            # MLP layer, feature-major fp32r (fp22 reads), 512 edges.
            # act_eng: 'scalar' | 'vector' | 'mixed' (t=0 scalar, t=1 vector
            # so the two half-activations run concurrently)
            # scale: optional AP multiplied into the pre-bias value (used to
            # fold *beta into the last hcn layer; its bias is b3*beta)
            def lin_h(src, wname, bname, relu, dst, act_eng="scalar",
                      scale=None):
                w, bias = w_sb[wname], b_sb[bname]
                for t in range(2):
                    pm = pmp.tile([128, EB], FP32, tag="pm")
                    for k in range(2):
                        nc.tensor.matmul(
                            pm[:], w[:, k, t * 128:(t + 1) * 128],
                            src[:, k, :], start=(k == 0), stop=(k == 1))
                    dsl = dst[:, t, :]
                    use_scalar = (act_eng == "scalar"
                                  or (act_eng == "mixed" and t == 0))
                    if use_scalar:
                        nc.scalar.activation(
                            dsl, pm[:], AF.Relu if relu else AF.Identity,
                            bias=bias[:, t, :],
                            scale=scale if scale is not None else 1.0)
                    elif scale is not None:
                        nc.vector.tensor_scalar(
                            dsl, pm[:], scale, bias[:, t, :],
                            ALU.mult, ALU.add)
                    elif relu:
                        nc.vector.tensor_scalar(
                            dsl, pm[:], bias[:, t, :], 0.0,
                            ALU.add, ALU.max)
                    else:
                        nc.vector.tensor_scalar_add(dsl, pm[:], bias[:, t, :])
                return dst

            # ---- per-block state ---------------------------------------
            v0f = {}     # flat AND-ed packed view per block
            xv = {}      # bf16 x row views per block
            pT = {}      # xi*xj product tiles
            px = {}      # xcn PSUM pairs
            om = {}      # expansion tiles per (block, mp)
            xij = {}     # xij MLP results
            u_t = {}     # xij hidden

            def prep_block(b):
                ga = [gt[(b, 0)], gt[(b, 1)]]
                # cn_packed = a0 AND a1 (adj byte-chunks 0..9 only, in place)
                v0 = ga[0][:].bitcast(I16)[:, 0:2 * NC5, :]
                v1 = ga[1][:].bitcast(I16)[:, 0:2 * NC5, :]
                nc.vector.tensor_tensor(v0, v0, v1, ALU.bitwise_and)
                v0f[b] = v0.rearrange("p a b -> p (a b)")
                xv[b] = [t[:].bitcast(BF16)[:, 2 * NC5:2 * NC5 + 4, :]
                         .rearrange("p (f s) w -> p f (s w)", f=2) for t in ga]

            def expand(b, mp):
                o = expp.tile([128, 2, NC5, 2 * EB], FP8, tag="exp",
                              name=f"om{b}_{mp}")
                o16 = o[:].bitcast(I16)
                for t in range(2):
                    m = 2 * mp + t
                    dst = o16[:, t, :, :].rearrange("p c e -> p (c e)")
                    if m < 4:
                        nc.vector.tensor_scalar(
                            dst, v0f[b], 4 - m, 0x1010,
                            ALU.logical_shift_left, ALU.bitwise_and)
                    elif m == 4:
                        nc.vector.tensor_scalar(
                            dst, v0f[b], 0x1010, None, ALU.bitwise_and)
                    else:
                        nc.vector.tensor_scalar(
                            dst, v0f[b], m - 4, 0x1010,
                            ALU.logical_shift_right, ALU.bitwise_and)
                om[(b, mp)] = o

            def mm_group(b, mp):
                va = None if (b == 0 and mp == 0) else om[(b, mp)][:]
                for c in range(NC5):
                    base = omc[c][:] if va is None else va[:, :, c, :]
                    for par in range(2):
                        mov = (base
                               .rearrange("p t (i two) -> p t two i", two=2)
                               [:, :, par, :])
                        for fh in range(2):
                            nc.tensor.matmul(
                                px[b][fh][:],
                                x8n_sb[:, mp, c, par, fh, :, :],
                                mov,
                                start=(mp == 0 and c == 0 and par == 0),
                                stop=(mp == 3 and c == NC5 - 1 and par == 1),
                                perf_mode=mybir.MatmulPerfMode.DoubleRow)

            # ---- DVE prep: both blocks' AND/mult/expand queued before any
            # MM-dependent vector work so the MM streams never stall -------
            # block0 mp0 pipelined PER WORD-CHUNK: AND each chunk
            # out-of-place into its own small tile (chunk-granular deps)
            # and expand planes m=0,1 per chunk, so the first 4 matmuls of
            # chunk c can issue ~1us after g2's DMA instead of waiting for
            # the full AND + full-plane expansion (~3us).
            ga0, ga1 = gt[(0, 0)], gt[(0, 1)]
            v0 = ga0[:].bitcast(I16)[:, 0:2 * NC5, :]
            v1 = ga1[:].bitcast(I16)[:, 0:2 * NC5, :]
            omc = []
            for c in range(NC5):
                cc = expp.tile([128, 2, 256], I16, tag="cnc", bufs=NC5,
                               name=f"cnc{c}")
                nc.vector.tensor_tensor(cc[:], v0[:, 2 * c:2 * c + 2, :],
                                        v1[:, 2 * c:2 * c + 2, :],
                                        ALU.bitwise_and)
                oc = expp.tile([128, 2, 2 * EB], FP8, tag="omc", bufs=NC5,
                               name=f"omc{c}")
                oc16 = oc[:].bitcast(I16)
                ccf = cc[:].rearrange("p a b -> p (a b)")
                for t in range(2):   # planes m = 0, 1 -> shift left 4, 3
                    nc.vector.tensor_scalar(
                        oc16[:, t, :], ccf, 4 - t, 0x1010,
                        ALU.logical_shift_left, ALU.bitwise_and)
                omc.append(oc)
            # full AND (in place, original v0 untouched by the above) for
            # the remaining planes' full-tile expansions
            nc.vector.tensor_tensor(v0, v0, v1, ALU.bitwise_and)
            v0f[0] = v0.rearrange("p a b -> p (a b)")
            xv[0] = [t[:].bitcast(BF16)[:, 2 * NC5:2 * NC5 + 4, :]
                     .rearrange("p (f s) w -> p f (s w)", f=2)
                     for t in (ga0, ga1)]
            expand(0, 1)
            # mult AFTER the mp1 planes: pT0 isn't needed until the
            # xij0 layer (~mm group 1), but mp1's planes gate the MM stream
            pT[0] = actp.tile([128, 2, EB], FP32R, tag="act", name="pT0")
            nc.vector.tensor_mul(pT[0][:], xv[0][0], xv[0][1])
            expand(0, 2)
            expand(0, 3)
            prep_block(1)
            pT[1] = actp.tile([128, 2, EB], FP32R, tag="act", name="pT1")
            nc.vector.tensor_mul(pT[1][:], xv[1][0], xv[1][1])
            for mp in range(4):
                expand(1, mp)

            def hcn_layer(b, li, act_eng="mixed"):
                wn, bn, rl = (("w1t", "b1", True), ("w2t", "b2", True))[li]
                src = xcn[b] if li == 0 else h[b]
                h[b] = lin_h(src, wn, bn, rl,
                             actp.tile([128, 2, EB], FP32R, tag="act",
                                       name=f"h{b}_{li}"),
                             act_eng=act_eng)

            pox = {}

            def xij_pre_final(b):
                # accumulate xij @ Wl into the final PSUM bank early; the h
                # MMs in combine_final extend the same accumulation group
                pox[b] = pop.tile([1, EB], FP32, tag="po", name=f"pox{b}")
                for k in range(2):
                    nc.tensor.matmul(pox[b][:], wlt_sb[:, k, :],
                                     xij[b][:, k, :], start=(k == 0),
                                     stop=False, skip_group_check=True)

            def combine_final(b):
                # hcn l3 + *beta are folded into wfold/bl (host); xij@Wl is
                # already accumulated in pox[b]
                po = pox[b]
                for k in range(2):
                    nc.tensor.matmul(po[:], wfold_sb[:, k, :], h[b][:, k, :],
                                     start=False, stop=(k == 1),
                                     skip_group_check=True)
                nc.scalar.activation(out_sb[:, b * EB:(b + 1) * EB],
                                     po[:], AF.Identity, bias=bl_sb)

            def xcn_copy(b):
                xcn[b] = actp.tile([128, 2, EB], FP32R, tag="act",
                                   name=f"xcn{b}")
                nc.scalar.activation(xcn[b][:, 0, :], px[b][0][:], AF.Copy)
                nc.vector.tensor_scalar(xcn[b][:, 1, :], px[b][1][:],
                                        1.0, 0.0, ALU.mult, ALU.add)

            # ---- Tensor stream: MM0 x80 (xij0 interleaved) --------------
            xcn, h = {}, {}
            px[0] = [pxp.tile([128, EB], FP32, tag="px", name=f"px0_{fh}")
                     for fh in range(2)]
            mm_group(0, 0)
            u_t[0] = lin_h(pT[0], "wat", "ba", True,
                           actp.tile([128, 2, EB], FP32R, tag="act",
                                     name="u0"))
            mm_group(0, 1)
            xij[0] = lin_h(u_t[0], "wbt", "bb", False,
                           actp.tile([128, 2, EB], FP32R, tag="act",
                                     name="xij0"))
            xij_pre_final(0)
            mm_group(0, 2)
            mm_group(0, 3)
            xcn_copy(0)

            # ---- MM1 x80 with xij1 + the whole hcn0 chain interleaved ---
            px[1] = [pxp.tile([128, EB], FP32, tag="px", name=f"px1_{fh}")
                     for fh in range(2)]
            mm_group(1, 0)
            u_t[1] = lin_h(pT[1], "wat", "ba", True,
                           actp.tile([128, 2, EB], FP32R, tag="act",
                                     name="u1"))
            hcn_layer(0, 0)
            mm_group(1, 1)
            xij[1] = lin_h(u_t[1], "wbt", "bb", False,
                           actp.tile([128, 2, EB], FP32R, tag="act",
                                     name="xij1"))
            xij_pre_final(1)
            hcn_layer(0, 1)
            mm_group(1, 2)
            mm_group(1, 3)
            combine_final(0)
            nc.sync.dma_start(out_d[:, 0:EB], out_sb[:, 0:EB])
            xcn_copy(1)

            # ---- tail: only hcn1 remains ------------------------------
            for li in range(2):
                hcn_layer(1, li)
            combine_final(1)

            nc.sync.dma_start(out_d[:, EB:EC], out_sb[:, EB:EC])

    nc.compile()
    return nc


def _wrap_idx(ids, num):
    """Pack indices for dma_gather: [128, num//16] int16, idx i at
    [i % 16, i // 16], replicated over the 8 groups of 16 partitions."""
    a = np.asarray(ids).astype(np.int16)
    w = a.reshape(num // 16, 16).T.copy()
    return np.ascontiguousarray(np.tile(w, (8, 1)))


def prepare_inputs(x, adj, edge, W1, b1, W2, b2, W3, b3, Wa, ba, Wb, bb,
                   Wl, bl, beta):
    x = np.asarray(x, np.float32)
    adj = np.asarray(adj, np.float32)
    edge = np.asarray(edge)

    # extended rows: [packed adj bits (1280B) | x bf16 (512B)]
    adjp = np.zeros((N, NPAD), np.uint8)
    adjp[:, :N] = (adj != 0)
    adjx8 = np.zeros((N, ROWB), np.uint8)
    adjx8[:, :PKB] = np.packbits(adjp, axis=1, bitorder="little")
    adjx8[:, PKB:] = np.ascontiguousarray(
        x.astype(BF16_NP)).view(np.uint8).reshape(N, 2 * D)
    adjx = adjx8.view(FP8_NP)

    # permuted + scaled fp8 x table, mp-major:
    # x8n[p, mp, c, par, fh, t, f] = 32*x[2048c + 16p + 8par + 2mp + t,
    #                                     fh*128 + f]
    x8 = np.zeros((NPAD, D), FP8_NP)
    x8[:N] = np.clip(x * XSCALE, -224.0, 224.0).astype(FP8_NP)
    p_, mp_, c_, par_, t_ = np.meshgrid(
        np.arange(128), np.arange(4), np.arange(NC5), np.arange(2),
        np.arange(2), indexing="ij")
    nodes = 2048 * c_ + 16 * p_ + 8 * par_ + 2 * mp_ + t_
    tbl = x8[nodes]                                    # [128,4,5,2,2,256]
    tbl = tbl.reshape(128, 4, NC5, 2, 2, 2, 128)       # t, fh, f
    tbl = np.ascontiguousarray(tbl.transpose(0, 1, 2, 3, 5, 4, 6))
    x8n = tbl.reshape(128, -1)

    # packed weights: per partition p (fp32 elems):
    #   [5 x (k,h)=2x256 transposed weights][wlt 2][5 x bias 2][bl][beta][pad]
    wpack = np.zeros((128, 2576), np.float32)
    for i, W in enumerate((Wa, Wb, W1, W2, W3)):
        wt = np.asarray(W, np.float32).T.reshape(2, 128, H)   # [k, p, h]
        wpack[:, i * 512:(i + 1) * 512] = wt.transpose(1, 0, 2).reshape(128, 512)
    wpack[:, 2560:2562] = np.asarray(Wl, np.float32).T.reshape(2, 128).T
    beta0 = float(np.asarray(beta, np.float64).reshape(-1)[0])
    # hcn l3 is linear: fold beta*(Wl @ W3) into the final projection and
    # beta*(Wl @ b3) into the output bias (float64, exact)
    wfold = (beta0 * (np.asarray(Wl, np.float64) @ np.asarray(W3, np.float64)))
    blfold = (np.asarray(bl, np.float64).reshape(-1)[0]
              + beta0 * float((np.asarray(Wl, np.float64)
                               @ np.asarray(b3, np.float64)).reshape(-1)[0]))
    for i, bv in enumerate((ba, bb, b1, b2)):
        wpack[:, 2562 + 2 * i:2564 + 2 * i] = \
            np.asarray(bv, np.float32).reshape(2, 128).T
    wpack[:, 2570:2572] = wfold.astype(np.float32).T.reshape(2, 128).T
    wpack[:, 2572] = np.float32(blfold)

    common = dict(adjx=adjx, x8n=x8n, wpack=wpack)
    in_maps = []
    for c in range(N_CORES):
        m = dict(common)
        gi = []
        for b in range(NB):
            sl = slice(c * EC + b * EB, c * EC + (b + 1) * EB)
            for s in range(2):
                gi.append(_wrap_idx(edge[sl, s], EB))
        m["idxg"] = np.ascontiguousarray(np.hstack(gi))
        in_maps.append(m)
    return in_maps


_CACHE = {}


def _get_program():
    if "nc" not in _CACHE:
        _CACHE["nc"] = build_program()
    return _CACHE["nc"]


def run(in_maps, **kw):
    nc = _get_program()
    return run_bass_kernel_spmd(nc, in_maps, list(range(N_CORES)), **kw)


def kernel(**inputs):
    in_maps = prepare_inputs(**inputs)
    res = run(in_maps)
    out = np.concatenate([res.results[c]["out"][0] for c in range(N_CORES)])
    return out.reshape(E, 1).astype(np.float32)


# revision 49
# speedup vs baseline: 1.0618x; 1.0165x over previous
"""CNLP (common-neighbor link prediction) kernel for Trainium2, 8 NeuronCores.

Reference computation (per query edge e = (i, j)):
    cn  = adj[i] * adj[j]                      # common-neighbor indicator [N]
    xcn = cn @ x                               # sum of common-neighbor feats
    xij = relu(x[i]*x[j] @ Wa.T + ba) @ Wb.T + bb
    hcn = (relu->relu->lin) 3-layer MLP on xcn
    out = (hcn * beta + xij) @ Wl.T + bl       # [E, 1]

Sharding: edges (E=8192) split 8 x 1024 across cores; adj/x/weights replicated.

Device strategy per core (1024 edges in 2 blocks of 512):
  - adj is binary -> BIT-PACKED host-side (10240 nodes -> 1280 bytes/row).
    Extended row [packed 1280B | x bf16 512B]; one gpsimd
    dma_gather(transpose=True) per (block, src).  The q7 library for the
    gather is preloaded with load_library(mlp) as the first pool
    instruction so its ~14us load overlaps the input DMAs.
  - DVE ANDs the packed pair (u16 2x mode) then EXPANDS bits to fp8 bytes
    with 8 fused shift+mask tensor_scalar ops per block:
        OUT[p, m, c, e] = shift_m(cn_packed[p, c, e]) & 0x1010
    giving fp8 byte 0x10 (=2^-5) at node 2048c + 16p + 8par + m (par = byte
    within the u16 lane).  The node permutation is absorbed into the
    host-permuted stationary x table (pre-scaled by 32: 2^-5 * 32x = x).
  - Big matmul FLIPPED: stationary = permuted fp8 x table, moving = expanded
    cn slices; PSUM accumulates xcn^T feature-major [128f, 512e].  DoubleRow
    fp8 perf mode (2 k-tiles = adjacent m-planes) for 2x PE rate.
  - Both blocks' big-MM streams run back-to-back (block1's AND/expand are
    queued on DVE before any MLP vector work); the xij 2-layer MLPs are
    interleaved INTO the MM streams; the two hcn tail chains run
    interleaved on separate act engines (scalar for b0, vector for b1).
"""

import numpy as np
import ml_dtypes

import concourse.bacc as bacc
import concourse.tile as tile
import concourse.mybir as mybir
from concourse import library_config
from concourse.bass_utils import run_bass_kernel_spmd

BF16 = mybir.dt.bfloat16
FP32 = mybir.dt.float32
FP32R = mybir.dt.float32r
FP8 = mybir.dt.float8e4
I16 = mybir.dt.int16
AF = mybir.ActivationFunctionType
ALU = mybir.AluOpType
BF16_NP = ml_dtypes.bfloat16
FP8_NP = ml_dtypes.float8_e4m3

N_CORES = 8
N, E, D, H = 10000, 8192, 256, 256
NPAD = 10240                      # n padded to a multiple of 2048
EC = E // N_CORES                 # 1024 edges per core
EB = 512                          # edges per block
NB = EC // EB                     # 2 blocks
PKB = NPAD // 8                   # 1280 packed adjacency bytes per row
ROWB = PKB + 2 * D                # 1792 bytes per extended row
NC5 = PKB // 256                  # 5 u16 word-chunks of packed bits
XSCALE = 32.0                     # x table pre-scale (cn byte is 2^-5)


def build_program():
    nc = bacc.Bacc("TRN2", target_bir_lowering=False, debug=False,
                   enable_asserts=False, num_devices=N_CORES)

    adjx = nc.dram_tensor("adjx", [N, ROWB], FP8, kind="ExternalInput")
    # permuted+scaled fp8 x table, mp-major: [p][mp][c][par][fh][t][f]
    x8n_d = nc.dram_tensor("x8n", [128, 4 * NC5 * 2 * 2 * 2 * 128], FP8,
                           kind="ExternalInput")
    idxg_d = nc.dram_tensor("idxg", [128, NB * 2 * EB // 16], I16,
                            kind="ExternalInput")
    wpack_d = nc.dram_tensor("wpack", [128, 2576], FP32, kind="ExternalInput")
    out_d = nc.dram_tensor("out", [1, EC], FP32, kind="ExternalOutput")

    with tile.TileContext(nc) as tc:
        # library load FIRST (before pool-init memsets): its ~13us q7 code
        # DMA gates the first gather and overlaps the input loads
        nc.gpsimd.load_library(library_config.mlp)
        with (
            tc.tile_pool(name="const", bufs=1) as constp,
            tc.tile_pool(name="gath", bufs=4) as gathp,
            tc.tile_pool(name="exp", bufs=8) as expp,
            tc.tile_pool(name="acts", bufs=10) as actp,
            tc.tile_pool(name="px", bufs=4, space="PSUM") as pxp,
            tc.tile_pool(name="pm", bufs=2, space="PSUM") as pmp,
            tc.tile_pool(name="po", bufs=2, space="PSUM") as pop,
        ):
            idxg_sb = constp.tile([128, NB * 2 * EB // 16], I16)
            nc.sync.dma_start(idxg_sb[:], idxg_d[:])

            # all 4 gathers upfront (pool queue; ~5us decode each, serial)
            gt = {}
            for b in range(NB):
                for s in range(2):
                    gsl = slice((2 * b + s) * EB // 16,
                                (2 * b + s + 1) * EB // 16)
                    t = gathp.tile([128, ROWB // 128, EB], FP8, tag="g",
                                   bufs=4, name=f"a{b}{s}")
                    nc.gpsimd.dma_gather(
                        t[:], adjx[:], idxg_sb[:, gsl], EB, EB,
                        elem_size=ROWB, transpose=True)
                    gt[(b, s)] = t

            # fp8 x table, split by mp quarter so early matmuls start sooner
            x8n_sb = constp.tile([128, 4, NC5, 2, 2, 2, 128], FP8)
            qsz = NC5 * 2 * 2 * 2 * 128
            for mp in range(4):
                nc.sync.dma_start(
                    x8n_sb[:, mp, :, :, :, :, :]
                    .rearrange("p c q f t g -> p (c q f t g)"),
                    x8n_d[:, mp * qsz:(mp + 1) * qsz])

            # weights: one packed DMA -> scalar-copy to fp32r (the BIR
            # verifier requires fp32r matmul operands to be fp32r-rounded)
            wpack = constp.tile([128, 2576], FP32)
            nc.sync.dma_start(wpack[:], wpack_d[:])
            w_sb = {}
            for i, nm in ((0, "wat"), (2, "w1t"), (3, "w2t")):
                t = constp.tile([128, 2, H], FP32R, tag=f"w_{nm}")
                nc.scalar.activation(
                    t[:], wpack[:, i * 512:(i + 1) * 512]
                    .rearrange("p (k h) -> p k h", k=2), AF.Copy)
                w_sb[nm] = t
            # xij layer 2 (Wb) is LINEAR too: wbfold = Wl @ Wb replaces
            # it; the pre-final PSUM feeds straight from relu(u) (exact)
            wbf_sb = constp.tile([128, 2, 1], FP32R)
            nc.scalar.activation(
                wbf_sb[:], wpack[:, 2564:2566]
                .rearrange("p (k o) -> p k o", k=2), AF.Copy)
            # folded final weights: hcn layer 3 is LINEAR, so
            # beta*(Wl @ W3) replaces it entirely (host-folded, exact)
            wfold_sb = constp.tile([128, 2, 1], FP32R)
            nc.scalar.activation(
                wfold_sb[:], wpack[:, 2570:2572]
                .rearrange("p (k o) -> p k o", k=2), AF.Copy)
            b_sb = {}
            for i, nm in ((0, "ba"), (2, "b1"), (3, "b2")):
                b_sb[nm] = (wpack[:, 2562 + 2 * i:2564 + 2 * i]
                            .rearrange("p (k o) -> p k o", k=2))
            bl_sb = wpack[0:1, 2572:2573]
            beta_sb = wpack[:, 2573:2574]

            out_sb = constp.tile([1, EC], FP32)

            # DVE warm-up: the first instruction of each ALU-op combo pays a
            # ~1.5-3us uop-program load; DVE is idle during the ~35us gather
            # startup, so prime every combo used later on a scrap tile.
            scrap = constp.tile([128, 32], I16)
            scrap2 = constp.tile([128, 32], FP32)
            nc.vector.tensor_tensor(scrap[:, 0:16], scrap[:, 0:16],
                                    scrap[:, 16:32], ALU.bitwise_and)
            nc.vector.tensor_scalar(scrap[:, 0:16], scrap[:, 16:32], 2,
                                    0x1010, ALU.logical_shift_left,
                                    ALU.bitwise_and)
            nc.vector.tensor_scalar(scrap[:, 0:16], scrap[:, 16:32], 2,
                                    0x1010, ALU.logical_shift_right,
                                    ALU.bitwise_and)
            nc.vector.tensor_scalar(scrap[:, 0:16], scrap[:, 16:32],
                                    0x1010, None, ALU.bitwise_and)
            nc.vector.tensor_scalar(scrap2[:, 0:16], scrap2[:, 16:32],
                                    1.0, 0.0, ALU.mult, ALU.add)
            nc.vector.tensor_scalar(scrap2[:, 0:16], scrap2[:, 16:32],
                                    0.0, 0.0, ALU.add, ALU.max)
            nc.vector.tensor_tensor(scrap2[:, 0:16], scrap2[:, 0:16],
                                    scrap2[:, 16:32], ALU.add)
            nc.vector.tensor_tensor(scrap2[:, 0:16], scrap2[:, 0:16],
                                    scrap2[:, 16:32], ALU.mult)

            # MLP layer, feature-major fp32r (fp22 reads), 512 edges.
            # act_eng: 'scalar' | 'vector' | 'mixed' (t=0 scalar, t=1 vector
            # so the two half-activations run concurrently)
            # scale: optional AP multiplied into the pre-bias value (used to
            # fold *beta into the last hcn layer; its bias is b3*beta)
            def lin_h(src, wname, bname, relu, dst, act_eng="scalar",
                      scale=None):
                w, bias = w_sb[wname], b_sb[bname]
                for t in range(2):
                    pm = pmp.tile([128, EB], FP32, tag="pm")
                    for k in range(2):
                        nc.tensor.matmul(
                            pm[:], w[:, k, t * 128:(t + 1) * 128],
                            src[:, k, :], start=(k == 0), stop=(k == 1))
                    dsl = dst[:, t, :]
                    use_scalar = (act_eng == "scalar"
                                  or (act_eng == "mixed" and t == 0))
                    if use_scalar:
                        nc.scalar.activation(
                            dsl, pm[:], AF.Relu if relu else AF.Identity,
                            bias=bias[:, t, :],
                            scale=scale if scale is not None else 1.0)
                    elif scale is not None:
                        nc.vector.tensor_scalar(
                            dsl, pm[:], scale, bias[:, t, :],
                            ALU.mult, ALU.add)
                    elif relu:
                        nc.vector.tensor_scalar(
                            dsl, pm[:], bias[:, t, :], 0.0,
                            ALU.add, ALU.max)
                    else:
                        nc.vector.tensor_scalar_add(dsl, pm[:], bias[:, t, :])
                return dst

            # ---- per-block state ---------------------------------------
            v0f = {}     # flat AND-ed packed view per block
            xv = {}      # bf16 x row views per block
            pT = {}      # xi*xj product tiles
            px = {}      # xcn PSUM pairs
            om = {}      # expansion tiles per (block, mp)
            xij = {}     # xij MLP results
            u_t = {}     # xij hidden

            def prep_block(b):
                ga = [gt[(b, 0)], gt[(b, 1)]]
                # cn_packed = a0 AND a1 (adj byte-chunks 0..9 only, in place)
                v0 = ga[0][:].bitcast(I16)[:, 0:2 * NC5, :]
                v1 = ga[1][:].bitcast(I16)[:, 0:2 * NC5, :]
                nc.vector.tensor_tensor(v0, v0, v1, ALU.bitwise_and)
                v0f[b] = v0.rearrange("p a b -> p (a b)")
                xv[b] = [t[:].bitcast(BF16)[:, 2 * NC5:2 * NC5 + 4, :]
                         .rearrange("p (f s) w -> p f (s w)", f=2) for t in ga]

            def expand(b, mp):
                o = expp.tile([128, 2, NC5, 2 * EB], FP8, tag="exp",
                              name=f"om{b}_{mp}")
                o16 = o[:].bitcast(I16)
                for t in range(2):
                    m = 2 * mp + t
                    dst = o16[:, t, :, :].rearrange("p c e -> p (c e)")
                    if m < 4:
                        nc.vector.tensor_scalar(
                            dst, v0f[b], 4 - m, 0x1010,
                            ALU.logical_shift_left, ALU.bitwise_and)
                    elif m == 4:
                        nc.vector.tensor_scalar(
                            dst, v0f[b], 0x1010, None, ALU.bitwise_and)
                    else:
                        nc.vector.tensor_scalar(
                            dst, v0f[b], m - 4, 0x1010,
                            ALU.logical_shift_right, ALU.bitwise_and)
                om[(b, mp)] = o

            def mm_group(b, mp):
                va = None if (b == 0 and mp == 0) else om[(b, mp)][:]
                for c in range(NC5):
                    base = omc[c][:] if va is None else va[:, :, c, :]
                    for par in range(2):
                        mov = (base
                               .rearrange("p t (i two) -> p t two i", two=2)
                               [:, :, par, :])
                        for fh in range(2):
                            nc.tensor.matmul(
                                px[b][fh][:],
                                x8n_sb[:, mp, c, par, fh, :, :],
                                mov,
                                start=(mp == 0 and c == 0 and par == 0),
                                stop=(mp == 3 and c == NC5 - 1 and par == 1),
                                perf_mode=mybir.MatmulPerfMode.DoubleRow)

            # ---- DVE prep: both blocks' AND/mult/expand queued before any
            # MM-dependent vector work so the MM streams never stall -------
            # block0 mp0 pipelined PER WORD-CHUNK: AND each chunk
            # out-of-place into its own small tile (chunk-granular deps)
            # and expand planes m=0,1 per chunk, so the first 4 matmuls of
            # chunk c can issue ~1us after g2's DMA instead of waiting for
            # the full AND + full-plane expansion (~3us).
            ga0, ga1 = gt[(0, 0)], gt[(0, 1)]
            v0 = ga0[:].bitcast(I16)[:, 0:2 * NC5, :]
            v1 = ga1[:].bitcast(I16)[:, 0:2 * NC5, :]
            omc = []
            for c in range(NC5):
                cc = expp.tile([128, 2, 256], I16, tag="cnc", bufs=NC5,
                               name=f"cnc{c}")
                nc.vector.tensor_tensor(cc[:], v0[:, 2 * c:2 * c + 2, :],
                                        v1[:, 2 * c:2 * c + 2, :],
                                        ALU.bitwise_and)
                oc = expp.tile([128, 2, 2 * EB], FP8, tag="omc", bufs=NC5,
                               name=f"omc{c}")
                oc16 = oc[:].bitcast(I16)
                ccf = cc[:].rearrange("p a b -> p (a b)")
                for t in range(2):   # planes m = 0, 1 -> shift left 4, 3
                    nc.vector.tensor_scalar(
                        oc16[:, t, :], ccf, 4 - t, 0x1010,
                        ALU.logical_shift_left, ALU.bitwise_and)
                omc.append(oc)
            # full AND (in place, original v0 untouched by the above) for
            # the remaining planes' full-tile expansions
            nc.vector.tensor_tensor(v0, v0, v1, ALU.bitwise_and)
            v0f[0] = v0.rearrange("p a b -> p (a b)")
            xv[0] = [t[:].bitcast(BF16)[:, 2 * NC5:2 * NC5 + 4, :]
                     .rearrange("p (f s) w -> p f (s w)", f=2)
                     for t in (ga0, ga1)]
            expand(0, 1)
            # mult AFTER the mp1 planes: pT0 isn't needed until the
            # xij0 layer (~mm group 1), but mp1's planes gate the MM stream
            pT[0] = actp.tile([128, 2, EB], FP32R, tag="act", name="pT0")
            nc.vector.tensor_mul(pT[0][:], xv[0][0], xv[0][1])
            expand(0, 2)
            expand(0, 3)
            prep_block(1)
            pT[1] = actp.tile([128, 2, EB], FP32R, tag="act", name="pT1")
            nc.vector.tensor_mul(pT[1][:], xv[1][0], xv[1][1])
            for mp in range(4):
                expand(1, mp)

            def hcn_layer(b, li, act_eng="mixed"):
                wn, bn, rl = (("w1t", "b1", True), ("w2t", "b2", True))[li]
                src = xcn[b] if li == 0 else h[b]
                h[b] = lin_h(src, wn, bn, rl,
                             actp.tile([128, 2, EB], FP32R, tag="act",
                                       name=f"h{b}_{li}"),
                             act_eng=act_eng)

            pox = {}

            def xij_pre_final(b):
                # accumulate xij @ Wl into the final PSUM bank early; the h
                # MMs in combine_final extend the same accumulation group
                pox[b] = pop.tile([1, EB], FP32, tag="po", name=f"pox{b}")
                for k in range(2):
                    nc.tensor.matmul(pox[b][:], wbf_sb[:, k, :],
                                     u_t[b][:, k, :], start=(k == 0),
                                     stop=False, skip_group_check=True)

            def combine_final(b):
                # hcn l3 + *beta are folded into wfold/bl (host); xij@Wl is
                # already accumulated in pox[b]
                po = pox[b]
                for k in range(2):
                    nc.tensor.matmul(po[:], wfold_sb[:, k, :], h[b][:, k, :],
                                     start=False, stop=(k == 1),
                                     skip_group_check=True)
                nc.scalar.activation(out_sb[:, b * EB:(b + 1) * EB],
                                     po[:], AF.Identity, bias=bl_sb)

            def xcn_copy(b):
                xcn[b] = actp.tile([128, 2, EB], FP32R, tag="act",
                                   name=f"xcn{b}")
                nc.scalar.activation(xcn[b][:, 0, :], px[b][0][:], AF.Copy)
                nc.vector.tensor_scalar(xcn[b][:, 1, :], px[b][1][:],
                                        1.0, 0.0, ALU.mult, ALU.add)

            # ---- Tensor stream: MM0 x80 (xij0 interleaved) --------------
            xcn, h = {}, {}
            px[0] = [pxp.tile([128, EB], FP32, tag="px", name=f"px0_{fh}")
                     for fh in range(2)]
            mm_group(0, 0)
            u_t[0] = lin_h(pT[0], "wat", "ba", True,
                           actp.tile([128, 2, EB], FP32R, tag="act",
                                     name="u0"))
            mm_group(0, 1)
            xij_pre_final(0)
            mm_group(0, 2)
            mm_group(0, 3)
            xcn_copy(0)

            # ---- MM1 x80 with xij1 + the whole hcn0 chain interleaved ---
            px[1] = [pxp.tile([128, EB], FP32, tag="px", name=f"px1_{fh}")
                     for fh in range(2)]
            mm_group(1, 0)
            u_t[1] = lin_h(pT[1], "wat", "ba", True,
                           actp.tile([128, 2, EB], FP32R, tag="act",
                                     name="u1"))
            hcn_layer(0, 0)
            mm_group(1, 1)
            xij_pre_final(1)
            hcn_layer(0, 1)
            mm_group(1, 2)
            mm_group(1, 3)
            combine_final(0)
            nc.sync.dma_start(out_d[:, 0:EB], out_sb[:, 0:EB])
            xcn_copy(1)

            # ---- tail: only hcn1 remains ------------------------------
            for li in range(2):
                hcn_layer(1, li)
            combine_final(1)

            nc.sync.dma_start(out_d[:, EB:EC], out_sb[:, EB:EC])

    nc.compile()
    return nc


def _wrap_idx(ids, num):
    """Pack indices for dma_gather: [128, num//16] int16, idx i at
    [i % 16, i // 16], replicated over the 8 groups of 16 partitions."""
    a = np.asarray(ids).astype(np.int16)
    w = a.reshape(num // 16, 16).T.copy()
    return np.ascontiguousarray(np.tile(w, (8, 1)))


def prepare_inputs(x, adj, edge, W1, b1, W2, b2, W3, b3, Wa, ba, Wb, bb,
                   Wl, bl, beta):
    x = np.asarray(x, np.float32)
    adj = np.asarray(adj, np.float32)
    edge = np.asarray(edge)

    # extended rows: [packed adj bits (1280B) | x bf16 (512B)]
    adjp = np.zeros((N, NPAD), np.uint8)
    adjp[:, :N] = (adj != 0)
    adjx8 = np.zeros((N, ROWB), np.uint8)
    adjx8[:, :PKB] = np.packbits(adjp, axis=1, bitorder="little")
    adjx8[:, PKB:] = np.ascontiguousarray(
        x.astype(BF16_NP)).view(np.uint8).reshape(N, 2 * D)
    adjx = adjx8.view(FP8_NP)

    # permuted + scaled fp8 x table, mp-major:
    # x8n[p, mp, c, par, fh, t, f] = 32*x[2048c + 16p + 8par + 2mp + t,
    #                                     fh*128 + f]
    x8 = np.zeros((NPAD, D), FP8_NP)
    x8[:N] = np.clip(x * XSCALE, -224.0, 224.0).astype(FP8_NP)
    p_, mp_, c_, par_, t_ = np.meshgrid(
        np.arange(128), np.arange(4), np.arange(NC5), np.arange(2),
        np.arange(2), indexing="ij")
    nodes = 2048 * c_ + 16 * p_ + 8 * par_ + 2 * mp_ + t_
    tbl = x8[nodes]                                    # [128,4,5,2,2,256]
    tbl = tbl.reshape(128, 4, NC5, 2, 2, 2, 128)       # t, fh, f
    tbl = np.ascontiguousarray(tbl.transpose(0, 1, 2, 3, 5, 4, 6))
    x8n = tbl.reshape(128, -1)

    # packed weights: per partition p (fp32 elems):
    #   [5 x (k,h)=2x256 transposed weights][wlt 2][5 x bias 2][bl][beta][pad]
    wpack = np.zeros((128, 2576), np.float32)
    for i, W in enumerate((Wa, Wb, W1, W2, W3)):
        wt = np.asarray(W, np.float32).T.reshape(2, 128, H)   # [k, p, h]
        wpack[:, i * 512:(i + 1) * 512] = wt.transpose(1, 0, 2).reshape(128, 512)
    wpack[:, 2560:2562] = np.asarray(Wl, np.float32).T.reshape(2, 128).T
    beta0 = float(np.asarray(beta, np.float64).reshape(-1)[0])
    # hcn l3 is linear: fold beta*(Wl @ W3) into the final projection and
    # beta*(Wl @ b3) into the output bias (float64, exact)
    Wl64 = np.asarray(Wl, np.float64)
    wfold = beta0 * (Wl64 @ np.asarray(W3, np.float64))
    wbfold = Wl64 @ np.asarray(Wb, np.float64)      # xij l2 fold (exact)
    blfold = (np.asarray(bl, np.float64).reshape(-1)[0]
              + beta0 * float((Wl64
                               @ np.asarray(b3, np.float64)).reshape(-1)[0])
              + float((Wl64 @ np.asarray(bb, np.float64)).reshape(-1)[0]))
    for i, bv in ((0, ba), (2, b1), (3, b2)):
        wpack[:, 2562 + 2 * i:2564 + 2 * i] = \
            np.asarray(bv, np.float32).reshape(2, 128).T
    wpack[:, 2564:2566] = wbfold.astype(np.float32).T.reshape(2, 128).T
    wpack[:, 2570:2572] = wfold.astype(np.float32).T.reshape(2, 128).T
    wpack[:, 2572] = np.float32(blfold)

    common = dict(adjx=adjx, x8n=x8n, wpack=wpack)
    in_maps = []
    for c in range(N_CORES):
        m = dict(common)
        gi = []
        for b in range(NB):
            sl = slice(c * EC + b * EB, c * EC + (b + 1) * EB)
            for s in range(2):
                gi.append(_wrap_idx(edge[sl, s], EB))
        m["idxg"] = np.ascontiguousarray(np.hstack(gi))
        in_maps.append(m)
    return in_maps


_CACHE = {}


def _get_program():
    if "nc" not in _CACHE:
        _CACHE["nc"] = build_program()
    return _CACHE["nc"]


def run(in_maps, **kw):
    nc = _get_program()
    return run_bass_kernel_spmd(nc, in_maps, list(range(N_CORES)), **kw)


def kernel(**inputs):
    in_maps = prepare_inputs(**inputs)
    res = run(in_maps)
    out = np.concatenate([res.results[c]["out"][0] for c in range(N_CORES)])
    return out.reshape(E, 1).astype(np.float32)
